# revision 42
# baseline (speedup 1.0000x reference)
"""Trainium2 Bass kernel for the additive-attention transformer.

Sharding: 8 cores = (batch b in 0..3) x (sequence half in 0..1).
Each core owns 128 query rows of one batch through 3 encoder layers.
After layers 1 and 2, core pairs AllGather the bf16-transposed updated
halves (single collective); natural-layout copies are rebuilt locally
with crossbar transpose-DMAs. The tiny layer-4 attention and the head
run on the host in fp32.

Scores use the tanh addition formula instead of materializing the
[Sq,Sk,H] feat tensor:
  tanh(q+k) = (tq+tk)/(1+tq*tk),  1/(1+u) = sum_m (-u)^m
  => scores[i,j] = sum_{m=0..M} sum_h A_m[h,i]*tk^m[h,j] + B_m[h,i]*tk^{m+1}[h,j]
  with A_m = (-1)^m wv tq^{m+1}, B_m = (-1)^m wv tq^m
i.e. 2(M+1) PSUM-accumulated rank-128 matmuls per layer.
"""

import numpy as np
import ml_dtypes

import concourse.bass as bass
import concourse.mybir as mybir
import concourse.tile as tile
from concourse import bacc
from concourse.bass_utils import run_bass_kernel_spmd
from concourse.masks import make_identity

F32 = mybir.dt.float32
BF16 = mybir.dt.bfloat16
AF = mybir.ActivationFunctionType
ALU = mybir.AluOpType

V, H, B, S = 1280, 128, 4, 256
P = 128          # partitions / own rows per core
VC = V // P      # 10 v-chunks
NCORES = 8
M = 5            # tanh-series truncation order
EPS = 1e-5

_CACHE = {}


def _build():
    nc = bacc.Bacc("TRN2", target_bir_lowering=False, debug=False,
                   num_devices=NCORES)

    # ---- I/O ----
    xo32_in = nc.dram_tensor("xo32", [P, V], F32, kind="ExternalInput")
    xot_in = nc.dram_tensor("xot", [P, VC, P], BF16, kind="ExternalInput")
    xfh_in = [nc.dram_tensor(f"xfh{r}", [P, V], BF16, kind="ExternalInput")
              for r in range(2)]
    w_in = {}
    for l in range(3):
        for kind in ("wq", "wk", "w1"):
            w_in[f"{kind}{l}"] = nc.dram_tensor(f"{kind}{l}", [P, VC, H], BF16,
                                                kind="ExternalInput")
        w_in[f"wvb1_{l}"] = nc.dram_tensor(f"wvb1_{l}", [P, 2], F32,
                                           kind="ExternalInput")
        w_in[f"w2{l}"] = nc.dram_tensor(f"w2{l}", [P, V], BF16,
                                        kind="ExternalInput")
        w_in[f"b2r{l}"] = nc.dram_tensor(f"b2r{l}", [1, V], BF16,
                                         kind="ExternalInput")
    zout = nc.dram_tensor("zout", [P, V], F32, kind="ExternalOutput")

    aginT = [nc.dram_tensor(f"aginT{l}", [P, VC * P], BF16) for l in range(2)]
    agoutT = [nc.dram_tensor(f"agoutT{l}", [2, P, VC * P], BF16)
              for l in range(2)]
    wuin = nc.dram_tensor("wuin", [1, 64], BF16)
    wuout = nc.dram_tensor("wuout", [2, 1, 64], BF16)
    groups = [[0, 1], [2, 3], [4, 5], [6, 7]]

    with tile.TileContext(nc) as tc:
        with tc.tile_pool(name="persist", bufs=1) as pp, \
             tc.tile_pool(name="xbuf", bufs=2) as xb, \
             tc.tile_pool(name="scratch", bufs=2) as sc, \
             tc.tile_pool(name="ps", bufs=1, space="PSUM") as ps, \
             tc.tile_pool(name="ps2", bufs=2, space="PSUM") as ps2:

            ident = pp.tile([P, P], BF16, tag="ident")
            make_identity(nc, ident[:])
            ones = pp.tile([P, 1], BF16, tag="ones")
            nc.vector.memset(ones[:], 1.0)
            ones128 = pp.tile([P, P], BF16, tag="ones128")
            nc.vector.memset(ones128[:], 1.0)
            ones256 = pp.tile([P, S], BF16, tag="ones256")
            nc.vector.memset(ones256[:], 1.0)
            onesrow = pp.tile([1, P], BF16, tag="onesrow")
            nc.vector.memset(onesrow[:], 1.0)

            # initial X + layer-0 q/k weights first (sync queue, in the order
            # layer 0 consumes them); everything else on the gpsimd queue.
            w = {}

            def _load_w(k, queue):
                t = w_in[k]
                tl = pp.tile(list(t.shape), t.dtype, tag=k)
                queue.dma_start(
                    out=tl[:], in_=t[(slice(None),) * len(t.shape)])
                w[k] = tl

            xot = xb.tile([P, VC, P], BF16, tag="xot")
            nc.sync.dma_start(xot[:], xot_in[:, :, :])
            _load_w("wq0", nc.scalar)
            xfh = []
            for r in range(2):
                t = xb.tile([P, V], BF16, tag=f"xfh{r}", name=f"xfh{r}_i")
                nc.sync.dma_start(t[:], xfh_in[r][:, :])
                xfh.append(t)
            _load_w("wk0", nc.sync)
            _load_w("wvb1_0", nc.sync)
            xo32 = xb.tile([P, V], F32, tag="xo32")
            nc.sync.dma_start(xo32[:], xo32_in[:, :])
            # transposed key halves derived on-device (SBUF->SBUF xbar)
            xfth = []
            for r in range(2):
                t = xb.tile([P, VC, P], BF16, tag=f"xfth{r}", name=f"xfth{r}_i")
                nc.scalar.dma_start_transpose(out=t[:], in_=xfh[r][:])
                xfth.append(t)
            for k in ("w10", "w20", "b2r0",
                      "wq1", "wk1", "wvb1_1", "w11", "w21", "b2r1",
                      "wq2", "wk2", "wvb1_2", "w12", "w22", "b2r2"):
                _load_w(k, nc.gpsimd)

            # warmup collective after the weight loads: initializes the CC
            # rings during layer-0 compute so the first real AllGather is
            # cheap, without head-blocking the weight DMAs.
            nc.gpsimd.collective_compute(
                "AllGather", ALU.bypass, replica_groups=groups,
                ins=[wuin[:, :]], outs=[wuout[:, :, :]])

            for l in range(3):
                wv = w[f"wvb1_{l}"][:, 0:1]
                b1 = w[f"wvb1_{l}"][:, 1:2]

                # ---- q/k projections (transposed layouts [h, i], [h, j]) ----
                qt_ps = ps.tile([P, P], F32, tag="qt")
                for c in range(VC):
                    nc.tensor.matmul(qt_ps[:], w[f"wq{l}"][:, c, :], xot[:, c, :],
                                     start=(c == 0), stop=(c == VC - 1))
                kt_ps = ps.tile([P, S], F32, tag="kt")
                for r in range(2):
                    for c in range(VC):
                        nc.tensor.matmul(kt_ps[:, r * P:(r + 1) * P],
                                         w[f"wk{l}"][:, c, :], xfth[r][:, c, :],
                                         start=(c == 0), stop=(c == VC - 1))

                tq = sc.tile([P, P], BF16, tag="tq")
                nc.scalar.activation(out=tq[:], in_=qt_ps[:], func=AF.Tanh)
                vn = sc.tile([P, P], BF16, tag="vn")
                nc.scalar.activation(out=vn[:], in_=qt_ps[:], func=AF.Tanh,
                                     scale=-1.0)
                tk = sc.tile([P, S], BF16, tag="tk")
                nc.scalar.activation(out=tk[:], in_=kt_ps[:], func=AF.Tanh)

                # ---- series feature maps (stride-2 chains for short deps) ----
                # A_m = (-1)^m wv tq^{m+1} ; B_m = (-1)^m wv tq^m ; pk_m = tk^m
                A = [sc.tile([P, P], BF16, tag=f"A{m}", name=f"A{m}_{l}")
                     for m in range(M + 1)]
                Bt = [sc.tile([P, P], BF16, tag=f"B{m}", name=f"B{m}_{l}")
                      for m in range(M + 1)]
                vn2 = sc.tile([P, P], BF16, tag="vn2")
                nc.vector.tensor_mul(out=vn2[:], in0=vn[:], in1=vn[:])
                nc.vector.tensor_scalar(out=A[0][:], in0=tq[:],
                                        scalar1=wv, scalar2=None, op0=ALU.mult)
                nc.vector.tensor_scalar(out=Bt[0][:], in0=ones128[:],
                                        scalar1=wv, scalar2=None, op0=ALU.mult)
                nc.vector.tensor_mul(out=A[1][:], in0=A[0][:], in1=vn[:])
                nc.vector.tensor_mul(out=Bt[1][:], in0=Bt[0][:], in1=vn[:])
                for m in range(2, M + 1):
                    nc.vector.tensor_mul(out=A[m][:], in0=A[m - 2][:], in1=vn2[:])
                    nc.vector.tensor_mul(out=Bt[m][:], in0=Bt[m - 2][:], in1=vn2[:])
                pk = [None] * (M + 2)
                pk[0] = ones256
                pk[1] = tk
                pk[2] = sc.tile([P, S], BF16, tag="pk2", name=f"pk2_{l}")
                nc.vector.tensor_mul(out=pk[2][:], in0=tk[:], in1=tk[:])
                for m in range(3, M + 2):
                    pk[m] = sc.tile([P, S], BF16, tag=f"pk{m}", name=f"pk{m}_{l}")
                    nc.vector.tensor_mul(out=pk[m][:], in0=pk[m - 2][:],
                                         in1=pk[2][:])

                # ---- scores[i,j] via 2(M+1) accumulated matmuls ----
                sc_ps = ps.tile([P, S], F32, tag="sc")
                for m in range(M + 1):
                    nc.tensor.matmul(sc_ps[:], A[m][:], pk[m][:],
                                     start=(m == 0), stop=False)
                    nc.tensor.matmul(sc_ps[:], Bt[m][:], pk[m + 1][:],
                                     start=False, stop=(m == M))

                # ---- softmax (no max-sub; scores are small) ----
                expt = sc.tile([P, S], BF16, tag="expt")
                sums = sc.tile([P, 1], F32, tag="sums")
                nc.scalar.activation(out=expt[:], in_=sc_ps[:], func=AF.Exp,
                                     accum_out=sums[:])
                rin = sc.tile([P, 1], F32, tag="rin")
                nc.vector.reciprocal(rin[:], sums[:])

                # transpose exp -> [j, i] halves for attnV
                e_ps = ps2.tile([P, 2, P], BF16, tag="yt")
                for jh in range(2):
                    nc.tensor.transpose(e_ps[:, jh, :],
                                        expt[:, jh * P:(jh + 1) * P], ident[:])
                expT = sc.tile([P, 2, P], BF16, tag="expT")
                nc.vector.tensor_copy(expT[:], e_ps[:])

                # ---- attnV ----
                av = ps.tile([P, V], F32, tag="big")
                for off in range(0, V, 512):
                    n = min(512, V - off)
                    for jh in range(2):
                        nc.tensor.matmul(av[:, off:off + n], expT[:, jh, :],
                                         xfh[jh][:, off:off + n],
                                         start=(jh == 0), stop=(jh == 1))

                # ---- ax = av/sums + X, LN ----
                ax = sc.tile([P, V], F32, tag="ax")
                nc.vector.scalar_tensor_tensor(
                    out=ax[:], in0=av[:], scalar=rin[:], in1=xo32[:],
                    op0=ALU.mult, op1=ALU.add)

                stats = sc.tile([P, 3, 6], F32, tag="stats")
                axg = ax[:, 0:1024].rearrange("p (n s) -> p n s", s=512)
                for g in range(2):
                    nc.vector.bn_stats(out=stats[:, g, :], in_=axg[:, g, :])
                nc.vector.bn_stats(out=stats[:, 2, :], in_=ax[:, 1024:1280])
                mv = sc.tile([P, 2], F32, tag="mv")
                nc.vector.bn_aggr(out=mv[:], in_=stats[:])
                # rstd = 1/sqrt(var+eps): linear seed on var in [0.85,1.35]
                # + one Newton iteration (rel err ~2e-4).
                vv = sc.tile([P, 1], F32, tag="vv")
                nc.vector.tensor_scalar(out=vv[:], in0=mv[:, 1:2], scalar1=EPS,
                                        scalar2=None, op0=ALU.add)
                r0 = sc.tile([P, 1], F32, tag="r0")
                nc.vector.tensor_scalar(out=r0[:], in0=vv[:], scalar1=-0.448,
                                        scalar2=1.4559, op0=ALU.mult, op1=ALU.add)
                t1 = sc.tile([P, 1], F32, tag="t1")
                nc.vector.tensor_mul(out=t1[:], in0=vv[:], in1=r0[:])
                nc.vector.tensor_mul(out=t1[:], in0=t1[:], in1=r0[:])
                nc.vector.tensor_scalar(out=t1[:], in0=t1[:], scalar1=-0.5,
                                        scalar2=1.5, op0=ALU.mult, op1=ALU.add)
                r_ = sc.tile([P, 1], F32, tag="r_")
                nc.vector.tensor_mul(out=r_[:], in0=r0[:], in1=t1[:])
                # y32 (DVE) and yb (ACT, Identity(ax*r - m*r)) both from ax,
                # running in parallel on the two engines.
                nmr = sc.tile([P, 1], F32, tag="nmr")
                nc.vector.tensor_mul(out=nmr[:], in0=mv[:, 0:1], in1=r_[:])
                nc.vector.tensor_scalar(out=nmr[:], in0=nmr[:], scalar1=-1.0,
                                        scalar2=None, op0=ALU.mult)
                yb = sc.tile([P, V], BF16, tag="yb")
                nc.scalar.activation(out=yb[:], in_=ax[:], func=AF.Identity,
                                     bias=nmr[:], scale=r_[:])
                y32 = sc.tile([P, V], F32, tag="y32")
                nc.vector.tensor_scalar(out=y32[:], in0=ax[:], scalar1=mv[:, 0:1],
                                        scalar2=r_[:], op0=ALU.subtract, op1=ALU.mult)

                # ---- YT via PE transpose (batched PSUM->SBUF copies) ----
                ybt = sc.tile([P, VC, P], BF16, tag="ybt")
                for g in range(2):
                    yt_ps = ps2.tile([P, 5, P], BF16, tag="yt")
                    for cc in range(5):
                        c = g * 5 + cc
                        nc.tensor.transpose(yt_ps[:, cc, :],
                                            yb[:, c * P:(c + 1) * P], ident[:])
                    nc.vector.tensor_copy(ybt[:, g * 5:(g + 1) * 5, :], yt_ps[:])

                # ---- FFN (b2 folded in via rank-1 matmul) ----
                h1_ps = ps.tile([P, P], F32, tag="qt")
                for c in range(VC):
                    nc.tensor.matmul(h1_ps[:], w[f"w1{l}"][:, c, :], ybt[:, c, :],
                                     start=(c == 0), stop=(c == VC - 1))
                h1r = sc.tile([P, P], BF16, tag="h1r")
                nc.scalar.activation(out=h1r[:], in_=h1_ps[:], func=AF.Relu,
                                     bias=b1, scale=1.0)
                o2 = ps.tile([P, V], F32, tag="big")
                for off in range(0, V, 512):
                    n = min(512, V - off)
                    nc.tensor.matmul(o2[:, off:off + n], h1r[:],
                                     w[f"w2{l}"][:, off:off + n],
                                     start=True, stop=False)
                    nc.tensor.matmul(o2[:, off:off + n], onesrow[:],
                                     w[f"b2r{l}"][:, off:off + n],
                                     start=False, stop=True)
                if l == 2:
                    z32 = xb.tile([P, V], F32, tag="xo32")
                    nc.vector.tensor_add(out=z32[:], in0=o2[:], in1=y32[:])
                    nc.sync.dma_start(zout[:, :], z32[:])
                    break

                # zb (bf16, gates the AllGather) first; fp32 residual after —
                # it is only needed at the next layer's ax.
                zb = sc.tile([P, V], BF16, tag="zb")
                nc.vector.tensor_add(out=zb[:], in0=o2[:], in1=y32[:])
                z32 = xb.tile([P, V], F32, tag="xo32")
                nc.vector.tensor_add(out=z32[:], in0=o2[:], in1=y32[:])
                zbt = xb.tile([P, VC, P], BF16, tag="xot")
                for g in range(2):
                    zt_ps = ps2.tile([P, 5, P], BF16, tag="yt")
                    for cc in range(5):
                        c = g * 5 + cc
                        nc.tensor.transpose(zt_ps[:, cc, :],
                                            zb[:, c * P:(c + 1) * P], ident[:])
                    nc.vector.tensor_copy(zbt[:, g * 5:(g + 1) * 5, :], zt_ps[:])

                # ---- AllGather of transposed halves only ----
                nc.sync.dma_start(
                    aginT[l][:, :], zbt[:].rearrange("p c i -> p (c i)"))
                nc.gpsimd.collective_compute(
                    "AllGather", ALU.bypass, replica_groups=groups,
                    ins=[aginT[l][:, :]], outs=[agoutT[l][:, :, :]])
                xfth_n, xfh_n = [], []
                for r in range(2):
                    t = xb.tile([P, VC, P], BF16, tag=f"xfth{r}",
                                name=f"xfth{r}_{l}")
                    nc.sync.dma_start(
                        t[:], agoutT[l][r, :, :].rearrange("p (c i) -> p c i",
                                                           i=P))
                    xfth_n.append(t)
                for r in range(2):
                    t = xb.tile([P, V], BF16, tag=f"xfh{r}", name=f"xfh{r}_{l}")
                    nc.sync.dma_start_transpose(
                        out=t[:].rearrange("p (c i) -> p c i", i=P),
                        in_=xfth_n[r][:].rearrange("p c i -> p (c i)"))
                    xfh_n.append(t)
                xo32, xot, xfth, xfh = z32, zbt, xfth_n, xfh_n

    nc.compile()
    return nc


def _bf(a):
    return np.ascontiguousarray(a.astype(ml_dtypes.bfloat16))


def kernel(**inputs):
    X = np.asarray(inputs["X"], dtype=np.float32)
    lys = int(np.asarray(inputs["lys_pos"]))
    if "nc" not in _CACHE:
        _CACHE["nc"] = _build()
    nc = _CACHE["nc"]

    # host-side prearranged per-core inputs
    wshared = {}
    for l, li in enumerate((1, 2, 3)):
        Wq = np.asarray(inputs[f"Wq{li}"], np.float32)
        Wk = np.asarray(inputs[f"Wk{li}"], np.float32)
        W1 = np.asarray(inputs[f"rW1_{li}"], np.float32)
        W2 = np.asarray(inputs[f"rW2_{li}"], np.float32)
        wshared[f"wq{l}"] = _bf(Wq.reshape(VC, P, H).transpose(1, 0, 2))
        wshared[f"wk{l}"] = _bf(Wk.reshape(VC, P, H).transpose(1, 0, 2))
        wshared[f"w1{l}"] = _bf(W1.reshape(VC, P, H).transpose(1, 0, 2))
        wv = np.asarray(inputs[f"wv{li}"], np.float32)
        b1 = np.asarray(inputs[f"rb1_{li}"], np.float32)
        wshared[f"wvb1_{l}"] = np.ascontiguousarray(
            np.stack([wv, b1], axis=1).astype(np.float32))
        wshared[f"w2{l}"] = _bf(W2)
        wshared[f"b2r{l}"] = _bf(
            np.asarray(inputs[f"rb2_{li}"], np.float32)[None, :])

    in_maps = []
    for c in range(NCORES):
        b, h = c // 2, c % 2
        Xb = X[b]                        # [S, V]
        Xo = Xb[h * P:(h + 1) * P]       # [P, V]
        m = dict(wshared)
        m["xo32"] = np.ascontiguousarray(Xo)
        m["xot"] = _bf(Xo.T.reshape(VC, P, P).transpose(1, 0, 2))
        for r in range(2):
            m[f"xfh{r}"] = _bf(Xb[r * P:(r + 1) * P])
        in_maps.append(m)

    import os as _os
    _trace = bool(_os.environ.get("BASS_TRACE"))
    res = run_bass_kernel_spmd(
        nc, in_maps, core_ids=list(range(NCORES)),
        trace=_trace,
        tmpdir=_os.environ.get("KTRACE_DIR") if _trace else None,
        trace_cores=[0] if _trace else None)
    _CACHE["last_res"] = res

    X3 = np.zeros((B, S, V), np.float32)
    for c in range(NCORES):
        b, h = c // 2, c % 2
        X3[b, h * P:(h + 1) * P] = res.results[c]["zout"]

    # ---- layer 4 + head on host (fp32) ----
    def ln(x):
        m_ = x.mean(-1, keepdims=True)
        v_ = ((x - m_) ** 2).mean(-1, keepdims=True)
        return (x - m_) / np.sqrt(v_ + EPS)

    Wq4 = np.asarray(inputs["Wq4"], np.float32)
    Wk4 = np.asarray(inputs["Wk4"], np.float32)
    wv4 = np.asarray(inputs["wv4"], np.float32)
    Xl = X3[:, lys, :][:, None, :]                       # [B,1,V]
    q = Xl @ Wq4                                         # [B,1,H]
    k = X3 @ Wk4                                         # [B,S,H]
    feat = np.tanh(q[:, :, None, :] + k[:, None, :, :])  # [B,1,S,H]
    sco = np.einsum("bijh,h->bij", feat, wv4)
    sco = sco - sco.max(-1, keepdims=True)
    a = np.exp(sco)
    a /= a.sum(-1, keepdims=True)
    att = np.einsum("bij,bjd->bid", a, X3)
    Xl = ln(att + Xl)
    h_ = np.maximum(Xl @ np.asarray(inputs["hW1"], np.float32)
                    + np.asarray(inputs["hb1"], np.float32), 0.0)
    h_ = np.maximum(h_ @ np.asarray(inputs["hW2"], np.float32)
                    + np.asarray(inputs["hb2"], np.float32), 0.0)
    logits = (h_ @ np.asarray(inputs["hW3"], np.float32)
              + np.asarray(inputs["hb3"], np.float32))[:, 0, :]
    return logits.astype(np.float32)


# revision 43
# speedup vs baseline: 1.0063x; 1.0063x over previous
"""Trainium2 Bass kernel for the additive-attention transformer.

Sharding: 8 cores = (batch b in 0..3) x (sequence half in 0..1).
Each core owns 128 query rows of one batch through 3 encoder layers.
After layers 1 and 2, core pairs AllGather the bf16-transposed updated
halves (single collective); natural-layout copies are rebuilt locally
with crossbar transpose-DMAs. The tiny layer-4 attention and the head
run on the host in fp32.

Scores use the tanh addition formula instead of materializing the
[Sq,Sk,H] feat tensor:
  tanh(q+k) = (tq+tk)/(1+tq*tk),  1/(1+u) = sum_m (-u)^m
  => scores[i,j] = sum_{m=0..M} sum_h A_m[h,i]*tk^m[h,j] + B_m[h,i]*tk^{m+1}[h,j]
  with A_m = (-1)^m wv tq^{m+1}, B_m = (-1)^m wv tq^m
i.e. 2(M+1) PSUM-accumulated rank-128 matmuls per layer.
"""

import numpy as np
import ml_dtypes

import concourse.bass as bass
import concourse.mybir as mybir
import concourse.tile as tile
from concourse import bacc
from concourse.bass_utils import run_bass_kernel_spmd
from concourse.masks import make_identity

F32 = mybir.dt.float32
BF16 = mybir.dt.bfloat16
AF = mybir.ActivationFunctionType
ALU = mybir.AluOpType

V, H, B, S = 1280, 128, 4, 256
P = 128          # partitions / own rows per core
VC = V // P      # 10 v-chunks
NCORES = 8
M = 5            # tanh-series truncation order
EPS = 1e-5

_CACHE = {}


def _build():
    nc = bacc.Bacc("TRN2", target_bir_lowering=False, debug=False,
                   num_devices=NCORES)

    # ---- I/O ----
    xo32_in = nc.dram_tensor("xo32", [P, V], F32, kind="ExternalInput")
    xot_in = nc.dram_tensor("xot", [P, VC, P], BF16, kind="ExternalInput")
    xfh_in = [nc.dram_tensor(f"xfh{r}", [P, V], BF16, kind="ExternalInput")
              for r in range(2)]
    w_in = {}
    for l in range(3):
        for kind in ("wq", "wk", "w1"):
            w_in[f"{kind}{l}"] = nc.dram_tensor(f"{kind}{l}", [P, VC, H], BF16,
                                                kind="ExternalInput")
        w_in[f"wvb1_{l}"] = nc.dram_tensor(f"wvb1_{l}", [P, 2], F32,
                                           kind="ExternalInput")
        w_in[f"w2{l}"] = nc.dram_tensor(f"w2{l}", [P, V], BF16,
                                        kind="ExternalInput")
        w_in[f"b2r{l}"] = nc.dram_tensor(f"b2r{l}", [1, V], BF16,
                                         kind="ExternalInput")
    zout = nc.dram_tensor("zout", [P, V], F32, kind="ExternalOutput")

    aginT = [nc.dram_tensor(f"aginT{l}", [P, VC * P], BF16) for l in range(2)]
    agoutT = [nc.dram_tensor(f"agoutT{l}", [2, P, VC * P], BF16)
              for l in range(2)]
    wuin = nc.dram_tensor("wuin", [1, 64], BF16)
    wuout = nc.dram_tensor("wuout", [2, 1, 64], BF16)
    groups = [[0, 1], [2, 3], [4, 5], [6, 7]]

    with tile.TileContext(nc) as tc:
        with tc.tile_pool(name="persist", bufs=1) as pp, \
             tc.tile_pool(name="xbuf", bufs=2) as xb, \
             tc.tile_pool(name="scratch", bufs=2) as sc, \
             tc.tile_pool(name="ps", bufs=1, space="PSUM") as ps, \
             tc.tile_pool(name="ps2", bufs=2, space="PSUM") as ps2:

            ident = pp.tile([P, P], BF16, tag="ident")
            make_identity(nc, ident[:])
            ones = pp.tile([P, 1], BF16, tag="ones")
            nc.vector.memset(ones[:], 1.0)
            ones128 = pp.tile([P, P], BF16, tag="ones128")
            nc.vector.memset(ones128[:], 1.0)
            ones256 = pp.tile([P, S], BF16, tag="ones256")
            nc.vector.memset(ones256[:], 1.0)
            onesrow = pp.tile([1, P], BF16, tag="onesrow")
            nc.vector.memset(onesrow[:], 1.0)

            # initial X + layer-0 q/k weights first (sync queue, in the order
            # layer 0 consumes them); everything else on the gpsimd queue.
            w = {}

            def _load_w(k, queue):
                t = w_in[k]
                tl = pp.tile(list(t.shape), t.dtype, tag=k)
                queue.dma_start(
                    out=tl[:], in_=t[(slice(None),) * len(t.shape)])
                w[k] = tl

            # split the two launch-critical loads across queues: each queue
            # moves ~160KB so qt's first chunks are ready in a few us
            xot = xb.tile([P, VC, P], BF16, tag="xot")
            nc.sync.dma_start(xot[:, 0:5, :], xot_in[:, 0:5, :])
            wq0t = pp.tile([P, VC, H], BF16, tag="wq0")
            nc.scalar.dma_start(wq0t[:, 0:5, :], w_in["wq0"][:, 0:5, :])
            nc.sync.dma_start(xot[:, 5:10, :], xot_in[:, 5:10, :])
            nc.scalar.dma_start(wq0t[:, 5:10, :], w_in["wq0"][:, 5:10, :])
            w["wq0"] = wq0t
            xfh = []
            for r in range(2):
                t = xb.tile([P, V], BF16, tag=f"xfh{r}", name=f"xfh{r}_i")
                nc.sync.dma_start(t[:], xfh_in[r][:, :])
                xfh.append(t)
            _load_w("wk0", nc.sync)
            _load_w("wvb1_0", nc.sync)
            xo32 = xb.tile([P, V], F32, tag="xo32")
            nc.sync.dma_start(xo32[:], xo32_in[:, :])
            # transposed key halves derived on-device (SBUF->SBUF xbar)
            xfth = []
            for r in range(2):
                t = xb.tile([P, VC, P], BF16, tag=f"xfth{r}", name=f"xfth{r}_i")
                nc.scalar.dma_start_transpose(out=t[:], in_=xfh[r][:])
                xfth.append(t)
            for k in ("w10", "w20", "b2r0",
                      "wq1", "wk1", "wvb1_1", "w11", "w21", "b2r1",
                      "wq2", "wk2", "wvb1_2", "w12", "w22", "b2r2"):
                _load_w(k, nc.gpsimd)

            # warmup collective after the weight loads: initializes the CC
            # rings during layer-0 compute so the first real AllGather is
            # cheap, without head-blocking the weight DMAs.
            nc.gpsimd.collective_compute(
                "AllGather", ALU.bypass, replica_groups=groups,
                ins=[wuin[:, :]], outs=[wuout[:, :, :]])

            for l in range(3):
                wv = w[f"wvb1_{l}"][:, 0:1]
                b1 = w[f"wvb1_{l}"][:, 1:2]

                # ---- q/k projections (transposed layouts [h, i], [h, j]) ----
                qt_ps = ps.tile([P, P], F32, tag="qt")
                for c in range(VC):
                    nc.tensor.matmul(qt_ps[:], w[f"wq{l}"][:, c, :], xot[:, c, :],
                                     start=(c == 0), stop=(c == VC - 1))
                kt_ps = ps.tile([P, S], F32, tag="kt")
                for r in range(2):
                    for c in range(VC):
                        nc.tensor.matmul(kt_ps[:, r * P:(r + 1) * P],
                                         w[f"wk{l}"][:, c, :], xfth[r][:, c, :],
                                         start=(c == 0), stop=(c == VC - 1))

                tq = sc.tile([P, P], BF16, tag="tq")
                nc.scalar.activation(out=tq[:], in_=qt_ps[:], func=AF.Tanh)
                vn = sc.tile([P, P], BF16, tag="vn")
                nc.scalar.activation(out=vn[:], in_=qt_ps[:], func=AF.Tanh,
                                     scale=-1.0)
                tk = sc.tile([P, S], BF16, tag="tk")
                nc.scalar.activation(out=tk[:], in_=kt_ps[:], func=AF.Tanh)

                # ---- series feature maps (stride-2 chains for short deps) ----
                # A_m = (-1)^m wv tq^{m+1} ; B_m = (-1)^m wv tq^m ; pk_m = tk^m
                A = [sc.tile([P, P], BF16, tag=f"A{m}", name=f"A{m}_{l}")
                     for m in range(M + 1)]
                Bt = [sc.tile([P, P], BF16, tag=f"B{m}", name=f"B{m}_{l}")
                      for m in range(M + 1)]
                vn2 = sc.tile([P, P], BF16, tag="vn2")
                nc.vector.tensor_mul(out=vn2[:], in0=vn[:], in1=vn[:])
                nc.vector.tensor_scalar(out=A[0][:], in0=tq[:],
                                        scalar1=wv, scalar2=None, op0=ALU.mult)
                nc.vector.tensor_scalar(out=Bt[0][:], in0=ones128[:],
                                        scalar1=wv, scalar2=None, op0=ALU.mult)
                nc.vector.tensor_mul(out=A[1][:], in0=A[0][:], in1=vn[:])
                nc.vector.tensor_mul(out=Bt[1][:], in0=Bt[0][:], in1=vn[:])
                for m in range(2, M + 1):
                    nc.vector.tensor_mul(out=A[m][:], in0=A[m - 2][:], in1=vn2[:])
                    nc.vector.tensor_mul(out=Bt[m][:], in0=Bt[m - 2][:], in1=vn2[:])
                pk = [None] * (M + 2)
                pk[0] = ones256
                pk[1] = tk
                pk[2] = sc.tile([P, S], BF16, tag="pk2", name=f"pk2_{l}")
                nc.vector.tensor_mul(out=pk[2][:], in0=tk[:], in1=tk[:])
                for m in range(3, M + 2):
                    pk[m] = sc.tile([P, S], BF16, tag=f"pk{m}", name=f"pk{m}_{l}")
                    nc.vector.tensor_mul(out=pk[m][:], in0=pk[m - 2][:],
                                         in1=pk[2][:])

                # ---- scores[i,j] via 2(M+1) accumulated matmuls ----
                sc_ps = ps.tile([P, S], F32, tag="sc")
                for m in range(M + 1):
                    nc.tensor.matmul(sc_ps[:], A[m][:], pk[m][:],
                                     start=(m == 0), stop=False)
                    nc.tensor.matmul(sc_ps[:], Bt[m][:], pk[m + 1][:],
                                     start=False, stop=(m == M))

                # ---- softmax (no max-sub; scores are small) ----
                expt = sc.tile([P, S], BF16, tag="expt")
                sums = sc.tile([P, 1], F32, tag="sums")
                nc.scalar.activation(out=expt[:], in_=sc_ps[:], func=AF.Exp,
                                     accum_out=sums[:])
                rin = sc.tile([P, 1], F32, tag="rin")
                nc.vector.reciprocal(rin[:], sums[:])

                # transpose exp -> [j, i] halves for attnV
                e_ps = ps2.tile([P, 2, P], BF16, tag="yt")
                for jh in range(2):
                    nc.tensor.transpose(e_ps[:, jh, :],
                                        expt[:, jh * P:(jh + 1) * P], ident[:])
                expT = sc.tile([P, 2, P], BF16, tag="expT")
                nc.vector.tensor_copy(expT[:], e_ps[:])

                # ---- attnV ----
                av = ps.tile([P, V], F32, tag="big")
                for off in range(0, V, 512):
                    n = min(512, V - off)
                    for jh in range(2):
                        nc.tensor.matmul(av[:, off:off + n], expT[:, jh, :],
                                         xfh[jh][:, off:off + n],
                                         start=(jh == 0), stop=(jh == 1))

                # ---- ax = av/sums + X, LN ----
                ax = sc.tile([P, V], F32, tag="ax")
                nc.vector.scalar_tensor_tensor(
                    out=ax[:], in0=av[:], scalar=rin[:], in1=xo32[:],
                    op0=ALU.mult, op1=ALU.add)

                stats = sc.tile([P, 3, 6], F32, tag="stats")
                axg = ax[:, 0:1024].rearrange("p (n s) -> p n s", s=512)
                for g in range(2):
                    nc.vector.bn_stats(out=stats[:, g, :], in_=axg[:, g, :])
                nc.vector.bn_stats(out=stats[:, 2, :], in_=ax[:, 1024:1280])
                mv = sc.tile([P, 2], F32, tag="mv")
                nc.vector.bn_aggr(out=mv[:], in_=stats[:])
                # rstd = 1/sqrt(var+eps): linear seed on var in [0.85,1.35]
                # + one Newton iteration (rel err ~2e-4).
                vv = sc.tile([P, 1], F32, tag="vv")
                nc.vector.tensor_scalar(out=vv[:], in0=mv[:, 1:2], scalar1=EPS,
                                        scalar2=None, op0=ALU.add)
                r0 = sc.tile([P, 1], F32, tag="r0")
                nc.vector.tensor_scalar(out=r0[:], in0=vv[:], scalar1=-0.448,
                                        scalar2=1.4559, op0=ALU.mult, op1=ALU.add)
                t1 = sc.tile([P, 1], F32, tag="t1")
                nc.vector.tensor_mul(out=t1[:], in0=vv[:], in1=r0[:])
                nc.vector.tensor_mul(out=t1[:], in0=t1[:], in1=r0[:])
                nc.vector.tensor_scalar(out=t1[:], in0=t1[:], scalar1=-0.5,
                                        scalar2=1.5, op0=ALU.mult, op1=ALU.add)
                r_ = sc.tile([P, 1], F32, tag="r_")
                nc.vector.tensor_mul(out=r_[:], in0=r0[:], in1=t1[:])
                # y32 (DVE) and yb (ACT, Identity(ax*r - m*r)) both from ax,
                # running in parallel on the two engines.
                nmr = sc.tile([P, 1], F32, tag="nmr")
                nc.vector.tensor_mul(out=nmr[:], in0=mv[:, 0:1], in1=r_[:])
                nc.vector.tensor_scalar(out=nmr[:], in0=nmr[:], scalar1=-1.0,
                                        scalar2=None, op0=ALU.mult)
                yb = sc.tile([P, V], BF16, tag="yb")
                nc.scalar.activation(out=yb[:], in_=ax[:], func=AF.Identity,
                                     bias=nmr[:], scale=r_[:])
                y32 = sc.tile([P, V], F32, tag="y32")
                nc.vector.tensor_scalar(out=y32[:], in0=ax[:], scalar1=mv[:, 0:1],
                                        scalar2=r_[:], op0=ALU.subtract, op1=ALU.mult)

                # ---- YT via PE transpose (batched PSUM->SBUF copies) ----
                ybt = sc.tile([P, VC, P], BF16, tag="ybt")
                for g in range(2):
                    yt_ps = ps2.tile([P, 5, P], BF16, tag="yt")
                    for cc in range(5):
                        c = g * 5 + cc
                        nc.tensor.transpose(yt_ps[:, cc, :],
                                            yb[:, c * P:(c + 1) * P], ident[:])
                    nc.vector.tensor_copy(ybt[:, g * 5:(g + 1) * 5, :], yt_ps[:])

                # ---- FFN (b2 folded in via rank-1 matmul) ----
                h1_ps = ps.tile([P, P], F32, tag="qt")
                for c in range(VC):
                    nc.tensor.matmul(h1_ps[:], w[f"w1{l}"][:, c, :], ybt[:, c, :],
                                     start=(c == 0), stop=(c == VC - 1))
                h1r = sc.tile([P, P], BF16, tag="h1r")
                nc.scalar.activation(out=h1r[:], in_=h1_ps[:], func=AF.Relu,
                                     bias=b1, scale=1.0)
                o2 = ps.tile([P, V], F32, tag="big")
                for off in range(0, V, 512):
                    n = min(512, V - off)
                    nc.tensor.matmul(o2[:, off:off + n], h1r[:],
                                     w[f"w2{l}"][:, off:off + n],
                                     start=True, stop=False)
                    nc.tensor.matmul(o2[:, off:off + n], onesrow[:],
                                     w[f"b2r{l}"][:, off:off + n],
                                     start=False, stop=True)
                if l == 2:
                    z32 = xb.tile([P, V], F32, tag="xo32")
                    nc.vector.tensor_add(out=z32[:], in0=o2[:], in1=y32[:])
                    nc.sync.dma_start(zout[:, :], z32[:])
                    break

                # zb (bf16, gates the AllGather) first; fp32 residual after —
                # it is only needed at the next layer's ax.
                zb = sc.tile([P, V], BF16, tag="zb")
                nc.vector.tensor_add(out=zb[:], in0=o2[:], in1=y32[:])
                z32 = xb.tile([P, V], F32, tag="xo32")
                nc.vector.tensor_add(out=z32[:], in0=o2[:], in1=y32[:])
                zbt = xb.tile([P, VC, P], BF16, tag="xot")
                for g in range(2):
                    zt_ps = ps2.tile([P, 5, P], BF16, tag="yt")
                    for cc in range(5):
                        c = g * 5 + cc
                        nc.tensor.transpose(zt_ps[:, cc, :],
                                            zb[:, c * P:(c + 1) * P], ident[:])
                    nc.vector.tensor_copy(zbt[:, g * 5:(g + 1) * 5, :], zt_ps[:])

                # ---- AllGather of transposed halves only ----
                nc.sync.dma_start(
                    aginT[l][:, :], zbt[:].rearrange("p c i -> p (c i)"))
                nc.gpsimd.collective_compute(
                    "AllGather", ALU.bypass, replica_groups=groups,
                    ins=[aginT[l][:, :]], outs=[agoutT[l][:, :, :]])
                xfth_n, xfh_n = [], []
                for r in range(2):
                    t = xb.tile([P, VC, P], BF16, tag=f"xfth{r}",
                                name=f"xfth{r}_{l}")
                    nc.sync.dma_start(
                        t[:], agoutT[l][r, :, :].rearrange("p (c i) -> p c i",
                                                           i=P))
                    xfth_n.append(t)
                for r in range(2):
                    t = xb.tile([P, V], BF16, tag=f"xfh{r}", name=f"xfh{r}_{l}")
                    nc.sync.dma_start_transpose(
                        out=t[:].rearrange("p (c i) -> p c i", i=P),
                        in_=xfth_n[r][:].rearrange("p c i -> p (c i)"))
                    xfh_n.append(t)
                xo32, xot, xfth, xfh = z32, zbt, xfth_n, xfh_n

    nc.compile()
    return nc


def _bf(a):
    return np.ascontiguousarray(a.astype(ml_dtypes.bfloat16))


def kernel(**inputs):
    X = np.asarray(inputs["X"], dtype=np.float32)
    lys = int(np.asarray(inputs["lys_pos"]))
    if "nc" not in _CACHE:
        _CACHE["nc"] = _build()
    nc = _CACHE["nc"]

    # host-side prearranged per-core inputs
    wshared = {}
    for l, li in enumerate((1, 2, 3)):
        Wq = np.asarray(inputs[f"Wq{li}"], np.float32)
        Wk = np.asarray(inputs[f"Wk{li}"], np.float32)
        W1 = np.asarray(inputs[f"rW1_{li}"], np.float32)
        W2 = np.asarray(inputs[f"rW2_{li}"], np.float32)
        wshared[f"wq{l}"] = _bf(Wq.reshape(VC, P, H).transpose(1, 0, 2))
        wshared[f"wk{l}"] = _bf(Wk.reshape(VC, P, H).transpose(1, 0, 2))
        wshared[f"w1{l}"] = _bf(W1.reshape(VC, P, H).transpose(1, 0, 2))
        wv = np.asarray(inputs[f"wv{li}"], np.float32)
        b1 = np.asarray(inputs[f"rb1_{li}"], np.float32)
        wshared[f"wvb1_{l}"] = np.ascontiguousarray(
            np.stack([wv, b1], axis=1).astype(np.float32))
        wshared[f"w2{l}"] = _bf(W2)
        wshared[f"b2r{l}"] = _bf(
            np.asarray(inputs[f"rb2_{li}"], np.float32)[None, :])

    in_maps = []
    for c in range(NCORES):
        b, h = c // 2, c % 2
        Xb = X[b]                        # [S, V]
        Xo = Xb[h * P:(h + 1) * P]       # [P, V]
        m = dict(wshared)
        m["xo32"] = np.ascontiguousarray(Xo)
        m["xot"] = _bf(Xo.T.reshape(VC, P, P).transpose(1, 0, 2))
        for r in range(2):
            m[f"xfh{r}"] = _bf(Xb[r * P:(r + 1) * P])
        in_maps.append(m)

    import os as _os
    _trace = bool(_os.environ.get("BASS_TRACE"))
    res = run_bass_kernel_spmd(
        nc, in_maps, core_ids=list(range(NCORES)),
        trace=_trace,
        tmpdir=_os.environ.get("KTRACE_DIR") if _trace else None,
        trace_cores=[0] if _trace else None)
    _CACHE["last_res"] = res

    X3 = np.zeros((B, S, V), np.float32)
    for c in range(NCORES):
        b, h = c // 2, c % 2
        X3[b, h * P:(h + 1) * P] = res.results[c]["zout"]

    # ---- layer 4 + head on host (fp32) ----
    def ln(x):
        m_ = x.mean(-1, keepdims=True)
        v_ = ((x - m_) ** 2).mean(-1, keepdims=True)
        return (x - m_) / np.sqrt(v_ + EPS)

    Wq4 = np.asarray(inputs["Wq4"], np.float32)
    Wk4 = np.asarray(inputs["Wk4"], np.float32)
    wv4 = np.asarray(inputs["wv4"], np.float32)
    Xl = X3[:, lys, :][:, None, :]                       # [B,1,V]
    q = Xl @ Wq4                                         # [B,1,H]
    k = X3 @ Wk4                                         # [B,S,H]
    feat = np.tanh(q[:, :, None, :] + k[:, None, :, :])  # [B,1,S,H]
    sco = np.einsum("bijh,h->bij", feat, wv4)
    sco = sco - sco.max(-1, keepdims=True)
    a = np.exp(sco)
    a /= a.sum(-1, keepdims=True)
    att = np.einsum("bij,bjd->bid", a, X3)
    Xl = ln(att + Xl)
    h_ = np.maximum(Xl @ np.asarray(inputs["hW1"], np.float32)
                    + np.asarray(inputs["hb1"], np.float32), 0.0)
    h_ = np.maximum(h_ @ np.asarray(inputs["hW2"], np.float32)
                    + np.asarray(inputs["hb2"], np.float32), 0.0)
    logits = (h_ @ np.asarray(inputs["hW3"], np.float32)
              + np.asarray(inputs["hb3"], np.float32))[:, 0, :]
    return logits.astype(np.float32)


# revision 47
# speedup vs baseline: 1.0155x; 1.0092x over previous
"""Trainium2 Bass kernel for the additive-attention transformer.

Sharding: 8 cores = (batch b in 0..3) x (sequence half in 0..1).
Each core owns 128 query rows of one batch through 3 encoder layers.
After layers 1 and 2, core pairs AllGather the bf16-transposed updated
halves (single collective); natural-layout copies are rebuilt locally
with crossbar transpose-DMAs. The tiny layer-4 attention and the head
run on the host in fp32.

Scores use the tanh addition formula instead of materializing the
[Sq,Sk,H] feat tensor:
  tanh(q+k) = (tq+tk)/(1+tq*tk),  1/(1+u) = sum_m (-u)^m
  => scores[i,j] = sum_{m=0..M} sum_h A_m[h,i]*tk^m[h,j] + B_m[h,i]*tk^{m+1}[h,j]
  with A_m = (-1)^m wv tq^{m+1}, B_m = (-1)^m wv tq^m
i.e. 2(M+1) PSUM-accumulated rank-128 matmuls per layer.
"""

import numpy as np
import ml_dtypes

import concourse.bass as bass
import concourse.mybir as mybir
import concourse.tile as tile
from concourse import bacc
from concourse.bass_utils import run_bass_kernel_spmd
from concourse.masks import make_identity

F32 = mybir.dt.float32
BF16 = mybir.dt.bfloat16
AF = mybir.ActivationFunctionType
ALU = mybir.AluOpType

V, H, B, S = 1280, 128, 4, 256
P = 128          # partitions / own rows per core
VC = V // P      # 10 v-chunks
NCORES = 8
M = 5            # tanh-series truncation order
EPS = 1e-5

_CACHE = {}


def _build():
    nc = bacc.Bacc("TRN2", target_bir_lowering=False, debug=False,
                   num_devices=NCORES)

    # ---- I/O ----
    xo32_in = nc.dram_tensor("xo32", [P, V], F32, kind="ExternalInput")
    xot_in = nc.dram_tensor("xot", [P, VC, P], BF16, kind="ExternalInput")
    xfh_in = [nc.dram_tensor(f"xfh{r}", [P, V], BF16, kind="ExternalInput")
              for r in range(2)]
    w_in = {}
    for l in range(3):
        for kind in ("wq", "wk", "w1"):
            w_in[f"{kind}{l}"] = nc.dram_tensor(f"{kind}{l}", [P, VC, H], BF16,
                                                kind="ExternalInput")
        w_in[f"wvb1_{l}"] = nc.dram_tensor(f"wvb1_{l}", [P, 2], F32,
                                           kind="ExternalInput")
        w_in[f"w2{l}"] = nc.dram_tensor(f"w2{l}", [P, V], BF16,
                                        kind="ExternalInput")
        w_in[f"b2r{l}"] = nc.dram_tensor(f"b2r{l}", [1, V], BF16,
                                         kind="ExternalInput")
    zout = nc.dram_tensor("zout", [P, V], F32, kind="ExternalOutput")

    aginT = [nc.dram_tensor(f"aginT{l}", [P, VC * P], BF16) for l in range(2)]
    agoutT = [nc.dram_tensor(f"agoutT{l}", [2, P, VC * P], BF16)
              for l in range(2)]
    wuin = nc.dram_tensor("wuin", [1, 64], BF16)
    wuout = nc.dram_tensor("wuout", [2, 1, 64], BF16)
    groups = [[0, 1], [2, 3], [4, 5], [6, 7]]

    with tile.TileContext(nc) as tc:
        with tc.tile_pool(name="persist", bufs=1) as pp, \
             tc.tile_pool(name="xbuf", bufs=2) as xb, \
             tc.tile_pool(name="scratch", bufs=2) as sc, \
             tc.tile_pool(name="ps", bufs=1, space="PSUM") as ps, \
             tc.tile_pool(name="ps2", bufs=2, space="PSUM") as ps2:

            ident = pp.tile([P, P], BF16, tag="ident")
            make_identity(nc, ident[:])
            ones = pp.tile([P, 1], BF16, tag="ones")
            nc.vector.memset(ones[:], 1.0)
            ones128 = pp.tile([P, P], BF16, tag="ones128")
            nc.vector.memset(ones128[:], 1.0)
            ones256 = pp.tile([P, S], BF16, tag="ones256")
            nc.vector.memset(ones256[:], 1.0)
            onesrow = pp.tile([1, P], BF16, tag="onesrow")
            nc.vector.memset(onesrow[:], 1.0)

            # initial X + layer-0 q/k weights first (sync queue, in the order
            # layer 0 consumes them); everything else on the gpsimd queue.
            w = {}

            def _load_w(k, queue):
                t = w_in[k]
                tl = pp.tile(list(t.shape), t.dtype, tag=k)
                queue.dma_start(
                    out=tl[:], in_=t[(slice(None),) * len(t.shape)])
                w[k] = tl

            # split the two launch-critical loads across queues: each queue
            # moves ~160KB so qt's first chunks are ready in a few us
            xot = xb.tile([P, VC, P], BF16, tag="xot")
            nc.sync.dma_start(xot[:, 0:5, :], xot_in[:, 0:5, :])
            wq0t = pp.tile([P, VC, H], BF16, tag="wq0")
            nc.scalar.dma_start(wq0t[:, 0:5, :], w_in["wq0"][:, 0:5, :])
            nc.sync.dma_start(xot[:, 5:10, :], xot_in[:, 5:10, :])
            nc.scalar.dma_start(wq0t[:, 5:10, :], w_in["wq0"][:, 5:10, :])
            w["wq0"] = wq0t
            xfh = []
            for r in range(2):
                t = xb.tile([P, V], BF16, tag=f"xfh{r}", name=f"xfh{r}_i")
                nc.sync.dma_start(t[:], xfh_in[r][:, :])
                xfh.append(t)
            _load_w("wk0", nc.sync)
            _load_w("wvb1_0", nc.sync)
            xo32 = xb.tile([P, V], F32, tag="xo32")
            nc.sync.dma_start(xo32[:], xo32_in[:, :])
            # transposed key halves derived on-device (SBUF->SBUF xbar)
            xfth = []
            for r in range(2):
                t = xb.tile([P, VC, P], BF16, tag=f"xfth{r}", name=f"xfth{r}_i")
                nc.scalar.dma_start_transpose(out=t[:], in_=xfh[r][:])
                xfth.append(t)
            for k in ("w10", "w20", "b2r0",
                      "wq1", "wk1", "wvb1_1", "w11", "w21", "b2r1",
                      "wq2", "wk2", "wvb1_2", "w12", "w22", "b2r2"):
                _load_w(k, nc.gpsimd)

            # warmup collective after the weight loads: initializes the CC
            # rings during layer-0 compute so the first real AllGather is
            # cheap, without head-blocking the weight DMAs.
            nc.gpsimd.collective_compute(
                "AllGather", ALU.bypass, replica_groups=groups,
                ins=[wuin[:, :]], outs=[wuout[:, :, :]])

            for l in range(3):
                wv = w[f"wvb1_{l}"][:, 0:1]
                b1 = w[f"wvb1_{l}"][:, 1:2]

                # ---- q/k projections (transposed layouts [h, i], [h, j]) ----
                qt_ps = ps.tile([P, P], F32, tag="qt")
                for c in range(VC):
                    nc.tensor.matmul(qt_ps[:], w[f"wq{l}"][:, c, :], xot[:, c, :],
                                     start=(c == 0), stop=(c == VC - 1))
                kt_ps = ps.tile([P, S], F32, tag="kt")
                for r in range(2):
                    for c in range(VC):
                        nc.tensor.matmul(kt_ps[:, r * P:(r + 1) * P],
                                         w[f"wk{l}"][:, c, :], xfth[r][:, c, :],
                                         start=(c == 0), stop=(c == VC - 1))

                tq = sc.tile([P, P], BF16, tag="tq")
                nc.scalar.activation(out=tq[:], in_=qt_ps[:], func=AF.Tanh)
                vn = sc.tile([P, P], BF16, tag="vn")
                nc.scalar.activation(out=vn[:], in_=qt_ps[:], func=AF.Tanh,
                                     scale=-1.0)
                tk = sc.tile([P, S], BF16, tag="tk")
                nc.scalar.activation(out=tk[:], in_=kt_ps[:], func=AF.Tanh)

                # ---- series feature maps (stride-2 chains for short deps) ----
                # A_m = (-1)^m wv tq^{m+1} ; B_m = (-1)^m wv tq^m ; pk_m = tk^m
                A = [sc.tile([P, P], BF16, tag=f"A{m}", name=f"A{m}_{l}")
                     for m in range(M + 1)]
                Bt = [sc.tile([P, P], BF16, tag=f"B{m}", name=f"B{m}_{l}")
                      for m in range(M + 1)]
                vn2 = sc.tile([P, P], BF16, tag="vn2")
                nc.vector.tensor_mul(out=vn2[:], in0=vn[:], in1=vn[:])
                nc.vector.tensor_scalar(out=A[0][:], in0=tq[:],
                                        scalar1=wv, scalar2=None, op0=ALU.mult)
                nc.vector.tensor_scalar(out=Bt[0][:], in0=ones128[:],
                                        scalar1=wv, scalar2=None, op0=ALU.mult)
                nc.vector.tensor_mul(out=A[1][:], in0=A[0][:], in1=vn[:])
                nc.vector.tensor_mul(out=Bt[1][:], in0=Bt[0][:], in1=vn[:])
                for m in range(2, M + 1):
                    nc.vector.tensor_mul(out=A[m][:], in0=A[m - 2][:], in1=vn2[:])
                    nc.vector.tensor_mul(out=Bt[m][:], in0=Bt[m - 2][:], in1=vn2[:])
                pk = [None] * (M + 2)
                pk[0] = ones256
                pk[1] = tk
                pk[2] = sc.tile([P, S], BF16, tag="pk2", name=f"pk2_{l}")
                nc.vector.tensor_mul(out=pk[2][:], in0=tk[:], in1=tk[:])
                for m in range(3, M + 2):
                    pk[m] = sc.tile([P, S], BF16, tag=f"pk{m}", name=f"pk{m}_{l}")
                    nc.vector.tensor_mul(out=pk[m][:], in0=pk[m - 2][:],
                                         in1=pk[2][:])

                # ---- scores[i,j] via 2(M+1) accumulated matmuls ----
                sc_ps = ps.tile([P, S], F32, tag="sc")
                for m in range(M + 1):
                    nc.tensor.matmul(sc_ps[:], A[m][:], pk[m][:],
                                     start=(m == 0), stop=False)
                    nc.tensor.matmul(sc_ps[:], Bt[m][:], pk[m + 1][:],
                                     start=False, stop=(m == M))

                # ---- softmax (no max-sub; scores are small) ----
                expt = sc.tile([P, S], BF16, tag="expt")
                sums = sc.tile([P, 1], F32, tag="sums")
                nc.scalar.activation(out=expt[:], in_=sc_ps[:], func=AF.Exp,
                                     accum_out=sums[:])
                rin = sc.tile([P, 1], F32, tag="rin")
                nc.vector.reciprocal(rin[:], sums[:])

                # transpose exp -> [j, i] halves for attnV
                e_ps = ps2.tile([P, 2, P], BF16, tag="yt")
                for jh in range(2):
                    nc.tensor.transpose(e_ps[:, jh, :],
                                        expt[:, jh * P:(jh + 1) * P], ident[:])
                expT = sc.tile([P, 2, P], BF16, tag="expT")
                nc.vector.tensor_copy(expT[:], e_ps[:])

                # ---- attnV / ax / bn_stats, chunk-pipelined across PE+DVE ----
                av = ps.tile([P, V], F32, tag="big")
                ax = sc.tile([P, V], F32, tag="ax")
                stats = sc.tile([P, 3, 6], F32, tag="stats")
                for g, off in enumerate(range(0, V, 512)):
                    n = min(512, V - off)
                    for jh in range(2):
                        nc.tensor.matmul(av[:, off:off + n], expT[:, jh, :],
                                         xfh[jh][:, off:off + n],
                                         start=(jh == 0), stop=(jh == 1))
                    nc.vector.scalar_tensor_tensor(
                        out=ax[:, off:off + n], in0=av[:, off:off + n],
                        scalar=rin[:], in1=xo32[:, off:off + n],
                        op0=ALU.mult, op1=ALU.add)
                    nc.vector.bn_stats(out=stats[:, g, :],
                                       in_=ax[:, off:off + n])
                mv = sc.tile([P, 2], F32, tag="mv")
                nc.vector.bn_aggr(out=mv[:], in_=stats[:])
                # rstd = 1/sqrt(var+eps): linear seed on var in [0.85,1.35]
                # + one Newton iteration (rel err ~2e-4).
                vv = sc.tile([P, 1], F32, tag="vv")
                nc.vector.tensor_scalar(out=vv[:], in0=mv[:, 1:2], scalar1=EPS,
                                        scalar2=None, op0=ALU.add)
                r0 = sc.tile([P, 1], F32, tag="r0")
                nc.vector.tensor_scalar(out=r0[:], in0=vv[:], scalar1=-0.448,
                                        scalar2=1.4559, op0=ALU.mult, op1=ALU.add)
                t1 = sc.tile([P, 1], F32, tag="t1")
                nc.vector.tensor_mul(out=t1[:], in0=vv[:], in1=r0[:])
                nc.vector.tensor_mul(out=t1[:], in0=t1[:], in1=r0[:])
                nc.vector.tensor_scalar(out=t1[:], in0=t1[:], scalar1=-0.5,
                                        scalar2=1.5, op0=ALU.mult, op1=ALU.add)
                r_ = sc.tile([P, 1], F32, tag="r_")
                nc.vector.tensor_mul(out=r_[:], in0=r0[:], in1=t1[:])
                # y32 (DVE) and yb (ACT, Identity(ax*r - m*r)) both from ax,
                # running in parallel on the two engines.
                nmr = sc.tile([P, 1], F32, tag="nmr")
                nc.vector.tensor_mul(out=nmr[:], in0=mv[:, 0:1], in1=r_[:])
                nc.vector.tensor_scalar(out=nmr[:], in0=nmr[:], scalar1=-1.0,
                                        scalar2=None, op0=ALU.mult)
                yb = sc.tile([P, V], BF16, tag="yb")
                nc.scalar.activation(out=yb[:], in_=ax[:], func=AF.Identity,
                                     bias=nmr[:], scale=r_[:])
                y32 = sc.tile([P, V], F32, tag="y32")
                nc.vector.tensor_scalar(out=y32[:], in0=ax[:], scalar1=mv[:, 0:1],
                                        scalar2=r_[:], op0=ALU.subtract, op1=ALU.mult)

                # ---- YT via PE transpose (batched PSUM->SBUF copies) ----
                ybt = sc.tile([P, VC, P], BF16, tag="ybt")
                for g in range(2):
                    yt_ps = ps2.tile([P, 5, P], BF16, tag="yt")
                    for cc in range(5):
                        c = g * 5 + cc
                        nc.tensor.transpose(yt_ps[:, cc, :],
                                            yb[:, c * P:(c + 1) * P], ident[:])
                    nc.vector.tensor_copy(ybt[:, g * 5:(g + 1) * 5, :], yt_ps[:])

                # ---- FFN (b2 folded in via rank-1 matmul) ----
                h1_ps = ps.tile([P, P], F32, tag="qt")
                for c in range(VC):
                    nc.tensor.matmul(h1_ps[:], w[f"w1{l}"][:, c, :], ybt[:, c, :],
                                     start=(c == 0), stop=(c == VC - 1))
                h1r = sc.tile([P, P], BF16, tag="h1r")
                nc.scalar.activation(out=h1r[:], in_=h1_ps[:], func=AF.Relu,
                                     bias=b1, scale=1.0)
                o2 = ps.tile([P, V], F32, tag="big")
                if l == 2:
                    # final layer: chunked z32 + zout DMAs for early finish
                    z32 = xb.tile([P, V], F32, tag="xo32")
                    for off in range(0, V, 512):
                        n = min(512, V - off)
                        nc.tensor.matmul(o2[:, off:off + n], h1r[:],
                                         w[f"w2{l}"][:, off:off + n],
                                         start=True, stop=False)
                        nc.tensor.matmul(o2[:, off:off + n], onesrow[:],
                                         w[f"b2r{l}"][:, off:off + n],
                                         start=False, stop=True)
                        nc.vector.tensor_add(out=z32[:, off:off + n],
                                             in0=o2[:, off:off + n],
                                             in1=y32[:, off:off + n])
                        q = (nc.sync, nc.scalar, nc.sync)[off // 512]
                        q.dma_start(zout[:, off:off + n],
                                    z32[:, off:off + n])
                    break

                # zb (bf16, gates the AllGather) chunk-pipelined behind the
                # o2 matmuls; fp32 residual z32 afterwards — only needed
                # at the next layer's ax.
                zb = sc.tile([P, V], BF16, tag="zb")
                for off in range(0, V, 512):
                    n = min(512, V - off)
                    nc.tensor.matmul(o2[:, off:off + n], h1r[:],
                                     w[f"w2{l}"][:, off:off + n],
                                     start=True, stop=False)
                    nc.tensor.matmul(o2[:, off:off + n], onesrow[:],
                                     w[f"b2r{l}"][:, off:off + n],
                                     start=False, stop=True)
                    nc.vector.tensor_add(out=zb[:, off:off + n],
                                         in0=o2[:, off:off + n],
                                         in1=y32[:, off:off + n])
                z32 = xb.tile([P, V], F32, tag="xo32")
                nc.vector.tensor_add(out=z32[:], in0=o2[:], in1=y32[:])
                zbt = xb.tile([P, VC, P], BF16, tag="xot")
                for g in range(2):
                    zt_ps = ps2.tile([P, 5, P], BF16, tag="yt")
                    for cc in range(5):
                        c = g * 5 + cc
                        nc.tensor.transpose(zt_ps[:, cc, :],
                                            zb[:, c * P:(c + 1) * P], ident[:])
                    nc.vector.tensor_copy(zbt[:, g * 5:(g + 1) * 5, :], zt_ps[:])

                # ---- AllGather of transposed halves only ----
                nc.sync.dma_start(
                    aginT[l][:, :], zbt[:].rearrange("p c i -> p (c i)"))
                nc.gpsimd.collective_compute(
                    "AllGather", ALU.bypass, replica_groups=groups,
                    ins=[aginT[l][:, :]], outs=[agoutT[l][:, :, :]])
                xfth_n, xfh_n = [], []
                for r in range(2):
                    t = xb.tile([P, VC, P], BF16, tag=f"xfth{r}",
                                name=f"xfth{r}_{l}")
                    nc.sync.dma_start(
                        t[:], agoutT[l][r, :, :].rearrange("p (c i) -> p c i",
                                                           i=P))
                    xfth_n.append(t)
                for r in range(2):
                    t = xb.tile([P, V], BF16, tag=f"xfh{r}", name=f"xfh{r}_{l}")
                    nc.sync.dma_start_transpose(
                        out=t[:].rearrange("p (c i) -> p c i", i=P),
                        in_=xfth_n[r][:].rearrange("p c i -> p (c i)"))
                    xfh_n.append(t)
                xo32, xot, xfth, xfh = z32, zbt, xfth_n, xfh_n

    nc.compile()
    return nc


def _bf(a):
    return np.ascontiguousarray(a.astype(ml_dtypes.bfloat16))


def kernel(**inputs):
    X = np.asarray(inputs["X"], dtype=np.float32)
    lys = int(np.asarray(inputs["lys_pos"]))
    if "nc" not in _CACHE:
        _CACHE["nc"] = _build()
    nc = _CACHE["nc"]

    # host-side prearranged per-core inputs
    wshared = {}
    for l, li in enumerate((1, 2, 3)):
        Wq = np.asarray(inputs[f"Wq{li}"], np.float32)
        Wk = np.asarray(inputs[f"Wk{li}"], np.float32)
        W1 = np.asarray(inputs[f"rW1_{li}"], np.float32)
        W2 = np.asarray(inputs[f"rW2_{li}"], np.float32)
        wshared[f"wq{l}"] = _bf(Wq.reshape(VC, P, H).transpose(1, 0, 2))
        wshared[f"wk{l}"] = _bf(Wk.reshape(VC, P, H).transpose(1, 0, 2))
        wshared[f"w1{l}"] = _bf(W1.reshape(VC, P, H).transpose(1, 0, 2))
        wv = np.asarray(inputs[f"wv{li}"], np.float32)
        b1 = np.asarray(inputs[f"rb1_{li}"], np.float32)
        wshared[f"wvb1_{l}"] = np.ascontiguousarray(
            np.stack([wv, b1], axis=1).astype(np.float32))
        wshared[f"w2{l}"] = _bf(W2)
        wshared[f"b2r{l}"] = _bf(
            np.asarray(inputs[f"rb2_{li}"], np.float32)[None, :])

    in_maps = []
    for c in range(NCORES):
        b, h = c // 2, c % 2
        Xb = X[b]                        # [S, V]
        Xo = Xb[h * P:(h + 1) * P]       # [P, V]
        m = dict(wshared)
        m["xo32"] = np.ascontiguousarray(Xo)
        m["xot"] = _bf(Xo.T.reshape(VC, P, P).transpose(1, 0, 2))
        for r in range(2):
            m[f"xfh{r}"] = _bf(Xb[r * P:(r + 1) * P])
        in_maps.append(m)

    import os as _os
    _trace = bool(_os.environ.get("BASS_TRACE"))
    res = run_bass_kernel_spmd(
        nc, in_maps, core_ids=list(range(NCORES)),
        trace=_trace,
        tmpdir=_os.environ.get("KTRACE_DIR") if _trace else None,
        trace_cores=[0] if _trace else None)
    _CACHE["last_res"] = res

    X3 = np.zeros((B, S, V), np.float32)
    for c in range(NCORES):
        b, h = c // 2, c % 2
        X3[b, h * P:(h + 1) * P] = res.results[c]["zout"]

    # ---- layer 4 + head on host (fp32) ----
    def ln(x):
        m_ = x.mean(-1, keepdims=True)
        v_ = ((x - m_) ** 2).mean(-1, keepdims=True)
        return (x - m_) / np.sqrt(v_ + EPS)

    Wq4 = np.asarray(inputs["Wq4"], np.float32)
    Wk4 = np.asarray(inputs["Wk4"], np.float32)
    wv4 = np.asarray(inputs["wv4"], np.float32)
    Xl = X3[:, lys, :][:, None, :]                       # [B,1,V]
    q = Xl @ Wq4                                         # [B,1,H]
    k = X3 @ Wk4                                         # [B,S,H]
    feat = np.tanh(q[:, :, None, :] + k[:, None, :, :])  # [B,1,S,H]
    sco = np.einsum("bijh,h->bij", feat, wv4)
    sco = sco - sco.max(-1, keepdims=True)
    a = np.exp(sco)
    a /= a.sum(-1, keepdims=True)
    att = np.einsum("bij,bjd->bid", a, X3)
    Xl = ln(att + Xl)
    h_ = np.maximum(Xl @ np.asarray(inputs["hW1"], np.float32)
                    + np.asarray(inputs["hb1"], np.float32), 0.0)
    h_ = np.maximum(h_ @ np.asarray(inputs["hW2"], np.float32)
                    + np.asarray(inputs["hb2"], np.float32), 0.0)
    logits = (h_ @ np.asarray(inputs["hW3"], np.float32)
              + np.asarray(inputs["hb3"], np.float32))[:, 0, :]
    return logits.astype(np.float32)


# revision 49
# speedup vs baseline: 1.0482x; 1.0322x over previous
"""Trainium2 Bass kernel for the additive-attention transformer.

Sharding: 8 cores = (batch b in 0..3) x (sequence half in 0..1).
Each core owns 128 query rows of one batch through 3 encoder layers.
After layers 1 and 2, core pairs AllGather the bf16-transposed updated
halves (single collective); natural-layout copies are rebuilt locally
with crossbar transpose-DMAs. The tiny layer-4 attention and the head
run on the host in fp32.

Scores use the tanh addition formula instead of materializing the
[Sq,Sk,H] feat tensor:
  tanh(q+k) = (tq+tk)/(1+tq*tk),  1/(1+u) = sum_m (-u)^m
  => scores[i,j] = sum_{m=0..M} sum_h A_m[h,i]*tk^m[h,j] + B_m[h,i]*tk^{m+1}[h,j]
  with A_m = (-1)^m wv tq^{m+1}, B_m = (-1)^m wv tq^m
i.e. 2(M+1) PSUM-accumulated rank-128 matmuls per layer.
"""

import numpy as np
import ml_dtypes

import concourse.bass as bass
import concourse.mybir as mybir
import concourse.tile as tile
from concourse import bacc
from concourse.bass_utils import run_bass_kernel_spmd
from concourse.masks import make_identity

F32 = mybir.dt.float32
BF16 = mybir.dt.bfloat16
AF = mybir.ActivationFunctionType
ALU = mybir.AluOpType

V, H, B, S = 1280, 128, 4, 256
P = 128          # partitions / own rows per core
VC = V // P      # 10 v-chunks
NCORES = 8
M = 5            # tanh-series truncation order
EPS = 1e-5

_CACHE = {}


def _build():
    nc = bacc.Bacc("TRN2", target_bir_lowering=False, debug=False,
                   num_devices=NCORES)

    # ---- I/O ----
    xo32_in = nc.dram_tensor("xo32", [P, V], F32, kind="ExternalInput")
    xot_in = nc.dram_tensor("xot", [P, VC, P], BF16, kind="ExternalInput")
    xfh_in = [nc.dram_tensor(f"xfh{r}", [P, V], BF16, kind="ExternalInput")
              for r in range(2)]
    w_in = {}
    for l in range(3):
        for kind in ("wq", "wk", "w1"):
            w_in[f"{kind}{l}"] = nc.dram_tensor(f"{kind}{l}", [P, VC, H], BF16,
                                                kind="ExternalInput")
        w_in[f"wvb1_{l}"] = nc.dram_tensor(f"wvb1_{l}", [P, 2], F32,
                                           kind="ExternalInput")
        w_in[f"w2{l}"] = nc.dram_tensor(f"w2{l}", [P, V], BF16,
                                        kind="ExternalInput")
        w_in[f"b2r{l}"] = nc.dram_tensor(f"b2r{l}", [1, V], BF16,
                                         kind="ExternalInput")
    zout = nc.dram_tensor("zout", [P, V], F32, kind="ExternalOutput")

    aginT = [nc.dram_tensor(f"aginT{l}", [P, VC * P], BF16) for l in range(2)]
    agoutT = [nc.dram_tensor(f"agoutT{l}", [2, P, VC * P], BF16)
              for l in range(2)]
    wuin = nc.dram_tensor("wuin", [1, 64], BF16)
    wuout = nc.dram_tensor("wuout", [2, 1, 64], BF16)
    groups = [[0, 1], [2, 3], [4, 5], [6, 7]]

    with tile.TileContext(nc) as tc:
        with tc.tile_pool(name="persist", bufs=1) as pp, \
             tc.tile_pool(name="xbuf", bufs=2) as xb, \
             tc.tile_pool(name="scratch", bufs=2) as sc, \
             tc.tile_pool(name="ps", bufs=1, space="PSUM") as ps, \
             tc.tile_pool(name="ps2", bufs=2, space="PSUM") as ps2:

            ident = pp.tile([P, P], BF16, tag="ident")
            make_identity(nc, ident[:])
            ones = pp.tile([P, 1], BF16, tag="ones")
            nc.vector.memset(ones[:], 1.0)
            ones128 = pp.tile([P, P], BF16, tag="ones128")
            nc.vector.memset(ones128[:], 1.0)
            ones256 = pp.tile([P, S], BF16, tag="ones256")
            nc.vector.memset(ones256[:], 1.0)
            onesrow = pp.tile([1, P], BF16, tag="onesrow")
            nc.vector.memset(onesrow[:], 1.0)

            # initial X + layer-0 q/k weights first (sync queue, in the order
            # layer 0 consumes them); everything else on the gpsimd queue.
            w = {}

            def _load_w(k, queue):
                t = w_in[k]
                tl = pp.tile(list(t.shape), t.dtype, tag=k)
                queue.dma_start(
                    out=tl[:], in_=t[(slice(None),) * len(t.shape)])
                w[k] = tl

            # per-queue DMA transfers serialize (~22.5GB/s each), so spread
            # layer-0-critical bytes across all three issue queues in
            # consumption order.
            xot = xb.tile([P, VC, P], BF16, tag="xot")
            wq0t = pp.tile([P, VC, H], BF16, tag="wq0")
            w["wq0"] = wq0t
            xfh = [xb.tile([P, V], BF16, tag=f"xfh{r}", name=f"xfh{r}_i")
                   for r in range(2)]
            xo32 = xb.tile([P, V], F32, tag="xo32")
            # sync: xot half, both natural halves (they feed the xbar
            # transposes, which must be fed from a HWDGE-loaded tile)
            nc.sync.dma_start(xot[:, 0:5, :], xot_in[:, 0:5, :])
            nc.sync.dma_start(xfh[0][:], xfh_in[0][:, :])
            nc.sync.dma_start(xfh[1][:], xfh_in[1][:, :])
            _load_w("wvb1_0", nc.sync)
            # scalar: q weights, then the xbar transposes, then xo32 half
            nc.scalar.dma_start(wq0t[:, 0:5, :], w_in["wq0"][:, 0:5, :])
            nc.scalar.dma_start(wq0t[:, 5:10, :], w_in["wq0"][:, 5:10, :])
            xfth = []
            for r in range(2):
                t = xb.tile([P, VC, P], BF16, tag=f"xfth{r}", name=f"xfth{r}_i")
                nc.scalar.dma_start_transpose(out=t[:], in_=xfh[r][:])
                xfth.append(t)
            nc.scalar.dma_start(xo32[:, 640:1280], xo32_in[:, 640:1280])
            # gpsimd: xo32 half, xot half, k weights, then the rest
            nc.gpsimd.dma_start(out=xo32[:, 0:640], in_=xo32_in[:, 0:640])
            nc.gpsimd.dma_start(out=xot[:, 5:10, :], in_=xot_in[:, 5:10, :])
            for k in ("wk0", "w10", "w20", "b2r0",
                      "wq1", "wk1", "wvb1_1", "w11", "w21", "b2r1",
                      "wq2", "wk2", "wvb1_2", "w12", "w22", "b2r2"):
                _load_w(k, nc.gpsimd)

            # warmup collective after the weight loads: initializes the CC
            # rings during layer-0 compute so the first real AllGather is
            # cheap, without head-blocking the weight DMAs.
            nc.gpsimd.collective_compute(
                "AllGather", ALU.bypass, replica_groups=groups,
                ins=[wuin[:, :]], outs=[wuout[:, :, :]])

            for l in range(3):
                wv = w[f"wvb1_{l}"][:, 0:1]
                b1 = w[f"wvb1_{l}"][:, 1:2]

                # ---- q/k projections (transposed layouts [h, i], [h, j]) ----
                qt_ps = ps.tile([P, P], F32, tag="qt")
                for c in range(VC):
                    nc.tensor.matmul(qt_ps[:], w[f"wq{l}"][:, c, :], xot[:, c, :],
                                     start=(c == 0), stop=(c == VC - 1))
                kt_ps = ps.tile([P, S], F32, tag="kt")
                for r in range(2):
                    for c in range(VC):
                        nc.tensor.matmul(kt_ps[:, r * P:(r + 1) * P],
                                         w[f"wk{l}"][:, c, :], xfth[r][:, c, :],
                                         start=(c == 0), stop=(c == VC - 1))

                tq = sc.tile([P, P], BF16, tag="tq")
                nc.scalar.activation(out=tq[:], in_=qt_ps[:], func=AF.Tanh)
                vn = sc.tile([P, P], BF16, tag="vn")
                nc.scalar.activation(out=vn[:], in_=qt_ps[:], func=AF.Tanh,
                                     scale=-1.0)
                tk = sc.tile([P, S], BF16, tag="tk")
                nc.scalar.activation(out=tk[:], in_=kt_ps[:], func=AF.Tanh)

                # ---- series feature maps (stride-2 chains for short deps) ----
                # A_m = (-1)^m wv tq^{m+1} ; B_m = (-1)^m wv tq^m ; pk_m = tk^m
                A = [sc.tile([P, P], BF16, tag=f"A{m}", name=f"A{m}_{l}")
                     for m in range(M + 1)]
                Bt = [sc.tile([P, P], BF16, tag=f"B{m}", name=f"B{m}_{l}")
                      for m in range(M + 1)]
                vn2 = sc.tile([P, P], BF16, tag="vn2")
                nc.vector.tensor_mul(out=vn2[:], in0=vn[:], in1=vn[:])
                nc.vector.tensor_scalar(out=A[0][:], in0=tq[:],
                                        scalar1=wv, scalar2=None, op0=ALU.mult)
                nc.vector.tensor_scalar(out=Bt[0][:], in0=ones128[:],
                                        scalar1=wv, scalar2=None, op0=ALU.mult)
                nc.vector.tensor_mul(out=A[1][:], in0=A[0][:], in1=vn[:])
                nc.vector.tensor_mul(out=Bt[1][:], in0=Bt[0][:], in1=vn[:])
                for m in range(2, M + 1):
                    nc.vector.tensor_mul(out=A[m][:], in0=A[m - 2][:], in1=vn2[:])
                    nc.vector.tensor_mul(out=Bt[m][:], in0=Bt[m - 2][:], in1=vn2[:])
                pk = [None] * (M + 2)
                pk[0] = ones256
                pk[1] = tk
                pk[2] = sc.tile([P, S], BF16, tag="pk2", name=f"pk2_{l}")
                nc.vector.tensor_mul(out=pk[2][:], in0=tk[:], in1=tk[:])
                for m in range(3, M + 2):
                    pk[m] = sc.tile([P, S], BF16, tag=f"pk{m}", name=f"pk{m}_{l}")
                    nc.vector.tensor_mul(out=pk[m][:], in0=pk[m - 2][:],
                                         in1=pk[2][:])

                # ---- scores[i,j] via 2(M+1) accumulated matmuls ----
                sc_ps = ps.tile([P, S], F32, tag="sc")
                for m in range(M + 1):
                    nc.tensor.matmul(sc_ps[:], A[m][:], pk[m][:],
                                     start=(m == 0), stop=False)
                    nc.tensor.matmul(sc_ps[:], Bt[m][:], pk[m + 1][:],
                                     start=False, stop=(m == M))

                # ---- softmax (no max-sub; scores are small) ----
                expt = sc.tile([P, S], BF16, tag="expt")
                sums = sc.tile([P, 1], F32, tag="sums")
                nc.scalar.activation(out=expt[:], in_=sc_ps[:], func=AF.Exp,
                                     accum_out=sums[:])
                rin = sc.tile([P, 1], F32, tag="rin")
                nc.vector.reciprocal(rin[:], sums[:])

                # transpose exp -> [j, i] halves for attnV
                e_ps = ps2.tile([P, 2, P], BF16, tag="yt")
                for jh in range(2):
                    nc.tensor.transpose(e_ps[:, jh, :],
                                        expt[:, jh * P:(jh + 1) * P], ident[:])
                expT = sc.tile([P, 2, P], BF16, tag="expT")
                nc.vector.tensor_copy(expT[:], e_ps[:])

                # ---- attnV / ax / bn_stats, chunk-pipelined across PE+DVE ----
                av = ps.tile([P, V], F32, tag="big")
                ax = sc.tile([P, V], F32, tag="ax")
                stats = sc.tile([P, 3, 6], F32, tag="stats")
                for g, off in enumerate(range(0, V, 512)):
                    n = min(512, V - off)
                    for jh in range(2):
                        nc.tensor.matmul(av[:, off:off + n], expT[:, jh, :],
                                         xfh[jh][:, off:off + n],
                                         start=(jh == 0), stop=(jh == 1))
                    nc.vector.scalar_tensor_tensor(
                        out=ax[:, off:off + n], in0=av[:, off:off + n],
                        scalar=rin[:], in1=xo32[:, off:off + n],
                        op0=ALU.mult, op1=ALU.add)
                    nc.vector.bn_stats(out=stats[:, g, :],
                                       in_=ax[:, off:off + n])
                mv = sc.tile([P, 2], F32, tag="mv")
                nc.vector.bn_aggr(out=mv[:], in_=stats[:])
                # rstd = 1/sqrt(var+eps): linear seed on var in [0.85,1.35]
                # + one Newton iteration (rel err ~2e-4).
                vv = sc.tile([P, 1], F32, tag="vv")
                nc.vector.tensor_scalar(out=vv[:], in0=mv[:, 1:2], scalar1=EPS,
                                        scalar2=None, op0=ALU.add)
                r0 = sc.tile([P, 1], F32, tag="r0")
                nc.vector.tensor_scalar(out=r0[:], in0=vv[:], scalar1=-0.448,
                                        scalar2=1.4559, op0=ALU.mult, op1=ALU.add)
                t1 = sc.tile([P, 1], F32, tag="t1")
                nc.vector.tensor_mul(out=t1[:], in0=vv[:], in1=r0[:])
                nc.vector.tensor_mul(out=t1[:], in0=t1[:], in1=r0[:])
                nc.vector.tensor_scalar(out=t1[:], in0=t1[:], scalar1=-0.5,
                                        scalar2=1.5, op0=ALU.mult, op1=ALU.add)
                r_ = sc.tile([P, 1], F32, tag="r_")
                nc.vector.tensor_mul(out=r_[:], in0=r0[:], in1=t1[:])
                # y32 (DVE) and yb (ACT, Identity(ax*r - m*r)) both from ax,
                # running in parallel on the two engines.
                nmr = sc.tile([P, 1], F32, tag="nmr")
                nc.vector.tensor_mul(out=nmr[:], in0=mv[:, 0:1], in1=r_[:])
                nc.vector.tensor_scalar(out=nmr[:], in0=nmr[:], scalar1=-1.0,
                                        scalar2=None, op0=ALU.mult)
                yb = sc.tile([P, V], BF16, tag="yb")
                nc.scalar.activation(out=yb[:], in_=ax[:], func=AF.Identity,
                                     bias=nmr[:], scale=r_[:])
                y32 = sc.tile([P, V], F32, tag="y32")
                nc.vector.tensor_scalar(out=y32[:], in0=ax[:], scalar1=mv[:, 0:1],
                                        scalar2=r_[:], op0=ALU.subtract, op1=ALU.mult)

                # ---- YT via PE transpose (batched PSUM->SBUF copies) ----
                ybt = sc.tile([P, VC, P], BF16, tag="ybt")
                for g in range(2):
                    yt_ps = ps2.tile([P, 5, P], BF16, tag="yt")
                    for cc in range(5):
                        c = g * 5 + cc
                        nc.tensor.transpose(yt_ps[:, cc, :],
                                            yb[:, c * P:(c + 1) * P], ident[:])
                    nc.vector.tensor_copy(ybt[:, g * 5:(g + 1) * 5, :], yt_ps[:])

                # ---- FFN (b2 folded in via rank-1 matmul) ----
                h1_ps = ps.tile([P, P], F32, tag="qt")
                for c in range(VC):
                    nc.tensor.matmul(h1_ps[:], w[f"w1{l}"][:, c, :], ybt[:, c, :],
                                     start=(c == 0), stop=(c == VC - 1))
                h1r = sc.tile([P, P], BF16, tag="h1r")
                nc.scalar.activation(out=h1r[:], in_=h1_ps[:], func=AF.Relu,
                                     bias=b1, scale=1.0)
                o2 = ps.tile([P, V], F32, tag="big")
                if l == 2:
                    # final layer: chunked z32 + zout DMAs for early finish
                    z32 = xb.tile([P, V], F32, tag="xo32")
                    for off in range(0, V, 512):
                        n = min(512, V - off)
                        nc.tensor.matmul(o2[:, off:off + n], h1r[:],
                                         w[f"w2{l}"][:, off:off + n],
                                         start=True, stop=False)
                        nc.tensor.matmul(o2[:, off:off + n], onesrow[:],
                                         w[f"b2r{l}"][:, off:off + n],
                                         start=False, stop=True)
                        nc.vector.tensor_add(out=z32[:, off:off + n],
                                             in0=o2[:, off:off + n],
                                             in1=y32[:, off:off + n])
                        q = (nc.sync, nc.scalar, nc.sync)[off // 512]
                        q.dma_start(zout[:, off:off + n],
                                    z32[:, off:off + n])
                    break

                # zb (bf16, gates the AllGather) chunk-pipelined behind the
                # o2 matmuls; fp32 residual z32 afterwards — only needed
                # at the next layer's ax.
                zb = sc.tile([P, V], BF16, tag="zb")
                for off in range(0, V, 512):
                    n = min(512, V - off)
                    nc.tensor.matmul(o2[:, off:off + n], h1r[:],
                                     w[f"w2{l}"][:, off:off + n],
                                     start=True, stop=False)
                    nc.tensor.matmul(o2[:, off:off + n], onesrow[:],
                                     w[f"b2r{l}"][:, off:off + n],
                                     start=False, stop=True)
                    nc.vector.tensor_add(out=zb[:, off:off + n],
                                         in0=o2[:, off:off + n],
                                         in1=y32[:, off:off + n])
                z32 = xb.tile([P, V], F32, tag="xo32")
                nc.vector.tensor_add(out=z32[:], in0=o2[:], in1=y32[:])
                zbt = xb.tile([P, VC, P], BF16, tag="xot")
                for g in range(2):
                    zt_ps = ps2.tile([P, 5, P], BF16, tag="yt")
                    for cc in range(5):
                        c = g * 5 + cc
                        nc.tensor.transpose(zt_ps[:, cc, :],
                                            zb[:, c * P:(c + 1) * P], ident[:])
                    nc.vector.tensor_copy(zbt[:, g * 5:(g + 1) * 5, :], zt_ps[:])

                # ---- AllGather of transposed halves only ----
                nc.sync.dma_start(
                    aginT[l][:, :], zbt[:].rearrange("p c i -> p (c i)"))
                nc.gpsimd.collective_compute(
                    "AllGather", ALU.bypass, replica_groups=groups,
                    ins=[aginT[l][:, :]], outs=[agoutT[l][:, :, :]])
                xfth_n, xfh_n = [], []
                for r in range(2):
                    t = xb.tile([P, VC, P], BF16, tag=f"xfth{r}",
                                name=f"xfth{r}_{l}")
                    nc.sync.dma_start(
                        t[:], agoutT[l][r, :, :].rearrange("p (c i) -> p c i",
                                                           i=P))
                    xfth_n.append(t)
                for r in range(2):
                    t = xb.tile([P, V], BF16, tag=f"xfh{r}", name=f"xfh{r}_{l}")
                    nc.sync.dma_start_transpose(
                        out=t[:].rearrange("p (c i) -> p c i", i=P),
                        in_=xfth_n[r][:].rearrange("p c i -> p (c i)"))
                    xfh_n.append(t)
                xo32, xot, xfth, xfh = z32, zbt, xfth_n, xfh_n

    nc.compile()
    return nc


def _bf(a):
    return np.ascontiguousarray(a.astype(ml_dtypes.bfloat16))


def kernel(**inputs):
    X = np.asarray(inputs["X"], dtype=np.float32)
    lys = int(np.asarray(inputs["lys_pos"]))
    if "nc" not in _CACHE:
        _CACHE["nc"] = _build()
    nc = _CACHE["nc"]

    # host-side prearranged per-core inputs
    wshared = {}
    for l, li in enumerate((1, 2, 3)):
        Wq = np.asarray(inputs[f"Wq{li}"], np.float32)
        Wk = np.asarray(inputs[f"Wk{li}"], np.float32)
        W1 = np.asarray(inputs[f"rW1_{li}"], np.float32)
        W2 = np.asarray(inputs[f"rW2_{li}"], np.float32)
        wshared[f"wq{l}"] = _bf(Wq.reshape(VC, P, H).transpose(1, 0, 2))
        wshared[f"wk{l}"] = _bf(Wk.reshape(VC, P, H).transpose(1, 0, 2))
        wshared[f"w1{l}"] = _bf(W1.reshape(VC, P, H).transpose(1, 0, 2))
        wv = np.asarray(inputs[f"wv{li}"], np.float32)
        b1 = np.asarray(inputs[f"rb1_{li}"], np.float32)
        wshared[f"wvb1_{l}"] = np.ascontiguousarray(
            np.stack([wv, b1], axis=1).astype(np.float32))
        wshared[f"w2{l}"] = _bf(W2)
        wshared[f"b2r{l}"] = _bf(
            np.asarray(inputs[f"rb2_{li}"], np.float32)[None, :])

    in_maps = []
    for c in range(NCORES):
        b, h = c // 2, c % 2
        Xb = X[b]                        # [S, V]
        Xo = Xb[h * P:(h + 1) * P]       # [P, V]
        m = dict(wshared)
        m["xo32"] = np.ascontiguousarray(Xo)
        m["xot"] = _bf(Xo.T.reshape(VC, P, P).transpose(1, 0, 2))
        for r in range(2):
            m[f"xfh{r}"] = _bf(Xb[r * P:(r + 1) * P])
        in_maps.append(m)

    import os as _os
    _trace = bool(_os.environ.get("BASS_TRACE"))
    res = run_bass_kernel_spmd(
        nc, in_maps, core_ids=list(range(NCORES)),
        trace=_trace,
        tmpdir=_os.environ.get("KTRACE_DIR") if _trace else None,
        trace_cores=[0] if _trace else None)
    _CACHE["last_res"] = res

    X3 = np.zeros((B, S, V), np.float32)
    for c in range(NCORES):
        b, h = c // 2, c % 2
        X3[b, h * P:(h + 1) * P] = res.results[c]["zout"]

    # ---- layer 4 + head on host (fp32) ----
    def ln(x):
        m_ = x.mean(-1, keepdims=True)
        v_ = ((x - m_) ** 2).mean(-1, keepdims=True)
        return (x - m_) / np.sqrt(v_ + EPS)

    Wq4 = np.asarray(inputs["Wq4"], np.float32)
    Wk4 = np.asarray(inputs["Wk4"], np.float32)
    wv4 = np.asarray(inputs["wv4"], np.float32)
    Xl = X3[:, lys, :][:, None, :]                       # [B,1,V]
    q = Xl @ Wq4                                         # [B,1,H]
    k = X3 @ Wk4                                         # [B,S,H]
    feat = np.tanh(q[:, :, None, :] + k[:, None, :, :])  # [B,1,S,H]
    sco = np.einsum("bijh,h->bij", feat, wv4)
    sco = sco - sco.max(-1, keepdims=True)
    a = np.exp(sco)
    a /= a.sum(-1, keepdims=True)
    att = np.einsum("bij,bjd->bid", a, X3)
    Xl = ln(att + Xl)
    h_ = np.maximum(Xl @ np.asarray(inputs["hW1"], np.float32)
                    + np.asarray(inputs["hb1"], np.float32), 0.0)
    h_ = np.maximum(h_ @ np.asarray(inputs["hW2"], np.float32)
                    + np.asarray(inputs["hb2"], np.float32), 0.0)
    logits = (h_ @ np.asarray(inputs["hW3"], np.float32)
              + np.asarray(inputs["hb3"], np.float32))[:, 0, :]
    return logits.astype(np.float32)
